# revision 17
# baseline (speedup 1.0000x reference)
"""EdgeGuidance Trainium2 kernel (v3).

Pipeline per image [3,544,960] -> [1,136,240]:
  gray = w.RGB  ->  smooth = gauss5x5(reflect)  ->  gx,gy = sobel(zero-pad)
  mag = sqrt(gx^2+gy^2+1e-6)  ->  4x4 avgpool  ->  sigmoid(5(x-0.2))^2

All linear steps fold into two banded-matrix passes on the PE (bf16
operands, fp32 PSUM):  gx = A_x @ gray @ Bx^T,  gy = A_y @ gray @ By^T.
Phase A uses gray as stationary so its output lands transposed ([w,s]);
phase B contracts over w with the B^T band as stationary.

Accuracy: gray is stored CENTERED (gray - c) in bf16, which also keeps
the phase-A output small in bf16. The centering constant re-enters as
rank-1 terms c*(B@1)(A@1)^T; the sobel bands' row-sums vanish in the
interior, so the correction is nonzero only on boundary rows/cols of
gx/gy and is applied with tiny tensor_adds into PSUM.

Input DMA: only full-128-partition 2-level loads (these spray across
all 16 SDMA engines; partial-partition or deeper patterns collapse
onto 2 engines/ring). Rows [0,256) and [236,492) as pair-of-row tiles
[128,1920], rows [416,544) as [128,960].

PSUM: single-bank tile rings (psa x4, psb x3, pooled x1 = 8 banks)
with chunk-staggered issue order so all engines pipeline.

Data parallel over batch: 8 cores x 2 images.
"""

import numpy as np
import ml_dtypes

import concourse.bass as bass
import concourse.tile as tile
from concourse import mybir
from concourse.bass_utils import run_bass_kernel_spmd

F32 = mybir.dt.float32
BF16 = mybir.dt.bfloat16
AF = mybir.ActivationFunctionType
ALU = mybir.AluOpType

B_FULL, C, H, W = 16, 3, 544, 960
N_CORES = 8
B_LOC = B_FULL // N_CORES
HP, WP = H // 4, W // 4  # 136, 240

BLUR_K, SIGMA = 5, 1.5
W_R, W_G, W_B = 0.2989, 0.587, 0.114
CEN = 1.672  # gray centering constant

SB = [(0, 250), (250, 486), (486, 544)]  # s-blocks; widths 250, 236, 58
N_WC = 8
BA_COLS = 2060


def _wj(j):
    return max(0, 120 * j - 3), min(W, 120 * j + 123)


# ---------------------------------------------------------------- numpy bands
def _blur1d():
    x = np.arange(BLUR_K, dtype=np.float64) - (BLUR_K - 1) / 2.0
    g = np.exp(-(x**2) / (2.0 * SIGMA**2))
    return g / g.sum()


def _band_reflect(n, taps):
    r = len(taps) // 2
    m = np.zeros((n, n), dtype=np.float64)
    for s in range(n):
        for d in range(-r, r + 1):
            i = s + d
            if i < 0:
                i = -i
            elif i >= n:
                i = 2 * n - 2 - i
            m[s, i] += taps[d + r]
    return m


def _band_zero(n, taps):
    r = len(taps) // 2
    m = np.zeros((n, n), dtype=np.float64)
    for s in range(n):
        for d in range(-r, r + 1):
            i = s + d
            if 0 <= i < n:
                m[s, i] += taps[d + r]
    return m


def _bands():
    g1 = _blur1d()
    vb = _band_reflect(H, g1)
    hb = _band_reflect(W, g1)
    ax = _band_zero(H, [1.0, 2.0, 1.0]) @ vb * W_R
    ay = _band_zero(H, [-1.0, 0.0, 1.0]) @ vb * W_R
    bx = _band_zero(W, [-1.0, 0.0, 1.0]) @ hb
    by = _band_zero(W, [1.0, 2.0, 1.0]) @ hb
    return ax, ay, bx, by


def build_constants():
    ax, ay, bx, by = _bands()

    band_a = np.zeros((128, BA_COLS), dtype=np.float64)
    # sb0: P0 rows 0..255 in pairs (q = row parity)
    s0, s1 = SB[0]
    w = s1 - s0
    for q in range(2):
        for p in range(128):
            r = 2 * p + q
            band_a[p, 500 * q : 500 * q + w] = ax[s0:s1, r]
            band_a[p, 500 * q + w : 500 * q + 2 * w] = ay[s0:s1, r]
    # sb1: P1 rows 236..491 in pairs
    s0, s1 = SB[1]
    w = s1 - s0
    for q in range(2):
        for p in range(128):
            r = 236 + 2 * p + q
            band_a[p, 1000 + 472 * q : 1000 + 472 * q + w] = ax[s0:s1, r]
            band_a[p, 1000 + 472 * q + w : 1000 + 472 * q + 2 * w] = ay[s0:s1, r]
    # sb2: P2 rows 416..543 (singles)
    s0, s1 = SB[2]
    w = s1 - s0
    for p in range(128):
        r = 416 + p
        band_a[p, 1944 : 1944 + w] = ax[s0:s1, r]
        band_a[p, 1944 + w : 1944 + 2 * w] = ay[s0:s1, r]

    band_b = np.zeros((128, 2 * N_WC * 120), dtype=np.float64)
    for t, m in enumerate((bx, by)):
        for j in range(N_WC):
            w0, w1 = _wj(j)
            blk = m[120 * j : 120 * j + 120, w0:w1].T
            band_b[0 : w1 - w0, (t * N_WC + j) * 120 : (t * N_WC + j + 1) * 120] = blk

    p4 = np.zeros((128, 30), dtype=np.float64)
    for wp in range(120):
        p4[wp, wp // 4] = 1.0 / 16.0

    # rank-1 centering corrections folded into the phase-B contraction:
    # band_b row 127 carries c*u (row-sums of B bands); xy row 127 carries
    # the v row-sums (of A bands), so the matmul adds c*u[w']*v[s].
    ux, vx = bx.sum(axis=1), ax.sum(axis=1)  # ux: +1 at w'=0, -1 at w'=959
    uy, vy = by.sum(axis=1), ay.sum(axis=1)  # vy: +W_R at s=0, -W_R at s=543
    for t, u in enumerate((ux, uy)):
        for j in range(N_WC):
            band_b[127, (t * N_WC + j) * 120 : (t * N_WC + j + 1) * 120] = (
                CEN * u[120 * j : 120 * j + 120]
            )
    cv = np.zeros((128, 1088), dtype=np.float32)
    for i, (s0, s1) in enumerate(SB):
        o = (0, 500, 972)[i]
        w = s1 - s0
        cv[127, o : o + w] = vx[s0:s1]
        cv[127, o + w : o + 2 * w] = vy[s0:s1]

    bf = ml_dtypes.bfloat16
    return band_a.astype(bf), band_b.astype(bf), p4.astype(bf), cv


# ------------------------------------------------------------------ bass build
def split_multi_waits(nc):
    """walrus in this container only accepts 1 sync-wait per instruction;
    hoist extra waits onto preceding same-engine NoOps."""
    for fn in nc.m.functions:
        for bb in fn.blocks:
            new_list, changed = [], False
            for ins in bb.instructions:
                si = ins.sync_info
                waits = list(si.on_wait) if si is not None else []
                if len(waits) > 1:
                    changed = True
                    for i, wt in enumerate(waits[:-1]):
                        new_list.append(
                            mybir.InstNoOp(
                                name=f"{ins.name}_ws{i}",
                                engine=ins.engine,
                                bass_nofuse=True,
                                sync_info=mybir.SyncInfo(on_wait=[wt], on_update=[]),
                            )
                        )
                    si.on_wait = [waits[-1]]
                    ins.sync_info = si
                new_list.append(ins)
            if changed:
                bb.instructions = new_list


def build_module():
    nc = bass.Bass("TRN2", target_bir_lowering=False, debug=False)
    x = nc.dram_tensor("x", [B_LOC, C, H, W], F32, kind="ExternalInput").ap()
    ba = nc.dram_tensor("bA", [128, BA_COLS], BF16, kind="ExternalInput").ap()
    bb_ = nc.dram_tensor("bB", [128, 2 * N_WC * 120], BF16, kind="ExternalInput").ap()
    p4 = nc.dram_tensor("p4", [128, 30], BF16, kind="ExternalInput").ap()
    cv = nc.dram_tensor("cv", [128, 1088], F32, kind="ExternalInput").ap()
    y = nc.dram_tensor("y", [B_LOC, 1, HP, WP], F32, kind="ExternalOutput").ap()
    xf = x.rearrange("b c h w -> (b c) (h w)")

    dmae = [nc.sync, nc.scalar]

    with tile.TileContext(nc) as tc:
        with (
            tc.tile_pool(name="const", bufs=1) as cpool,
            tc.tile_pool(name="rgb", bufs=1) as rgbp,
            tc.tile_pool(name="t1", bufs=2) as t1p,
            tc.tile_pool(name="gray", bufs=1) as grayp,
            tc.tile_pool(name="xy", bufs=3) as xyp,
            tc.tile_pool(name="sq", bufs=2) as sqp,
            tc.tile_pool(name="outp", bufs=2) as outp,
            tc.tile_pool(name="psa", bufs=4, space="PSUM") as psa,
            tc.tile_pool(name="psb", bufs=4, space="PSUM") as psb,
        ):
            # ---- constants
            ba_t = cpool.tile([128, BA_COLS], BF16, tag="ba")
            nc.scalar.dma_start(ba_t[:], ba[:])
            bb_t = cpool.tile([128, 2 * N_WC * 120], BF16, tag="bb")
            nc.scalar.dma_start(bb_t[:], bb_[:])
            p4_t = cpool.tile([128, 30], BF16, tag="p4")
            nc.scalar.dma_start(p4_t[:], p4[:])
            cv_t = cpool.tile([128, 1088], F32, tag="cv")
            nc.scalar.dma_start(cv_t[:], cv[:])
            bias_eps = cpool.tile([128, 1], F32, tag="beps")
            nc.gpsimd.memset(bias_eps[:], 1e-6)
            bias_m1 = cpool.tile([128, 1], F32, tag="bm1")
            nc.gpsimd.memset(bias_m1[:], -1.0)

            # ---- input DMAs: SWDGE (gpsimd) f32->bf16 cast loads, all
            # full-128-partition 2-level patterns (they spray across engines)
            rgb = {}
            for b in range(B_LOC):
                for blk, (r0, r1, fw) in enumerate(
                    [(0, 256, 1920), (236, 492, 1920), (416, 544, 960)]
                ):
                    for c in range(C):
                        t = rgbp.tile([128, fw], BF16, tag=f"rgb{b}{blk}{c}")
                        nc.gpsimd.dma_start(
                            t[:],
                            xf[3 * b + c, r0 * W : r1 * W].rearrange(
                                "(p f) -> p f", f=fw
                            ),
                        )
                        rgb[(b, blk, c)] = t

            gray = {}

            def emit_gray(b, blk):
                # centered gray: g = (R - c) + (wG/wR) G + (wB/wR) B, all bf16
                tr, tg, tb = (rgb[(b, blk, c)] for c in range(C))
                fw = 960 if blk == 2 else 1920
                with nc.allow_low_precision(reason="bf16 centered gray"):
                    nc.vector.tensor_scalar_sub(tr[:], tr[:], CEN)
                    t1 = t1p.tile([128, 1920], BF16, tag="t1")
                    nc.vector.scalar_tensor_tensor(
                        t1[0:128, 0:fw], tg[:], W_G / W_R, tr[:],
                        op0=ALU.mult, op1=ALU.add,
                    )
                    g = grayp.tile([128, fw], BF16, tag=f"g{b}{blk}")
                    nc.vector.scalar_tensor_tensor(
                        g[:], tb[:], W_B / W_R, t1[0:128, 0:fw],
                        op0=ALU.mult, op1=ALU.add,
                    )
                gray[(b, blk)] = g

            def stage_a(b, j, slot):
                g0, g1, g2 = gray[(b, 0)], gray[(b, 1)], gray[(b, 2)]
                w0, w1 = _wj(j)
                mj = w1 - w0
                a0 = psa.tile([128, 512], F32, tag="psa")
                nc.tensor.matmul(a0[0:mj, 0:500], g0[0:128, w0:w1],
                                 ba_t[0:128, 0:500], start=True, stop=False)
                nc.tensor.matmul(a0[0:mj, 0:500], g0[0:128, 960 + w0 : 960 + w1],
                                 ba_t[0:128, 500:1000], start=False, stop=True)
                a1 = psa.tile([128, 512], F32, tag="psa")
                nc.tensor.matmul(a1[0:mj, 0:472], g1[0:128, w0:w1],
                                 ba_t[0:128, 1000:1472], start=True, stop=False)
                nc.tensor.matmul(a1[0:mj, 0:472], g1[0:128, 960 + w0 : 960 + w1],
                                 ba_t[0:128, 1472:1944], start=False, stop=True)
                a2 = psa.tile([128, 512], F32, tag="psa")
                nc.tensor.matmul(a2[0:mj, 0:116], g2[0:128, w0:w1],
                                 ba_t[0:128, 1944:2060], start=True, stop=True)
                xy = xyp.tile([128, 1088], BF16, tag="xy")
                if slot < 3:
                    # rows 96:126 zeros + correction row 127; later ring reuses
                    # keep row 127 (drains only touch rows [0:126))
                    nc.vector.tensor_copy(xy[96:128, :], cv_t[96:128, :])
                nc.vector.tensor_copy(xy[0:mj, 0:500], a0[0:mj, 0:500])
                nc.scalar.copy(xy[0:mj, 500:736], a1[0:mj, 0:236])
                nc.vector.tensor_copy(xy[0:mj, 736:972], a1[0:mj, 236:472])
                nc.scalar.copy(xy[0:mj, 972:1088], a2[0:mj, 0:116])
                return xy

            def stage_b(b, j, xy, pooled):
                bTx = bb_t[0:128, (0 * N_WC + j) * 120 : (0 * N_WC + j + 1) * 120]
                bTy = bb_t[0:128, (1 * N_WC + j) * 120 : (1 * N_WC + j + 1) * 120]
                # b0: gx s[0:486]; b1: gy s[0:486]
                # b2: gx s[486:544] | gy s[486:544] | pool1 | pool2
                b0 = psb.tile([128, 512], F32, tag="psb")
                nc.tensor.matmul(b0[0:120, 0:250], bTx, xy[0:128, 0:250],
                                 start=True, stop=True)
                nc.tensor.matmul(b0[0:120, 250:486], bTx, xy[0:128, 500:736],
                                 start=True, stop=True)
                b1 = psb.tile([128, 512], F32, tag="psb")
                nc.tensor.matmul(b1[0:120, 0:250], bTy, xy[0:128, 250:500],
                                 start=True, stop=True)
                nc.tensor.matmul(b1[0:120, 250:486], bTy, xy[0:128, 736:972],
                                 start=True, stop=True)
                b2 = psb.tile([128, 512], F32, tag="psb")
                nc.tensor.matmul(b2[0:120, 0:58], bTx, xy[0:128, 972:1030],
                                 start=True, stop=True)
                nc.tensor.matmul(b2[0:120, 64:122], bTy, xy[0:128, 1030:1088],
                                 start=True, stop=True)

                # squares (f32), packed to s-order [120, 544]
                sqx = sqp.tile([128, 544], F32, tag="sqx")
                nc.scalar.activation(sqx[0:120, 0:486], b0[0:120, 0:486], AF.Square)
                nc.scalar.activation(sqx[0:120, 486:544], b2[0:120, 0:58], AF.Square)
                sqy = sqp.tile([128, 544], F32, tag="sqy")
                nc.scalar.activation(sqy[0:120, 0:486], b1[0:120, 0:486], AF.Square)
                nc.scalar.activation(sqy[0:120, 486:544], b2[0:120, 64:122], AF.Square)

                m2 = sqp.tile([128, 544], F32, tag="m2")
                nc.gpsimd.tensor_add(m2[0:120, :], sqx[0:120, :], sqy[0:120, :])
                mag = sqp.tile([128, 544], BF16, tag="mag")
                with nc.allow_low_precision(reason="bf16 magnitude"):
                    nc.scalar.activation(
                        mag[0:120, :], m2[0:120, :], AF.Sqrt, bias=bias_eps[0:120, :]
                    )
                    sp = sqp.tile([128, 136], BF16, tag="sp")
                    nc.vector.tensor_reduce(
                        sp[0:120, :],
                        mag[0:120, :].rearrange("p (g f) -> p g f", f=4),
                        axis=mybir.AxisListType.X,
                        op=ALU.add,
                    )
                nc.tensor.matmul(b2[0:96, 128:158], sp[0:120, 0:96], p4_t[0:120, :],
                                 start=True, stop=True)
                nc.tensor.matmul(b2[0:40, 192:222], sp[0:120, 96:136], p4_t[0:120, :],
                                 start=True, stop=True)
                nc.vector.tensor_copy(pooled[0:96, 30 * j : 30 * (j + 1)],
                                      b2[0:96, 128:158])
                nc.scalar.copy(pooled[0:40, WP + 30 * j : WP + 30 * (j + 1)],
                               b2[0:40, 192:222])

            def finish_image(b, pooled):
                sg = outp.tile([128, 2 * WP], F32, tag="sg")
                nc.scalar.activation(sg[0:96, 0:WP], pooled[0:96, 0:WP], AF.Sigmoid,
                                     bias=bias_m1[0:96, :], scale=5.0)
                nc.scalar.activation(sg[0:40, WP : 2 * WP],
                                     pooled[0:40, WP : 2 * WP], AF.Sigmoid,
                                     bias=bias_m1[0:40, :], scale=5.0)
                ot = outp.tile([128, 2 * WP], F32, tag="ot")
                nc.vector.tensor_mul(ot[0:96, 0:WP], sg[0:96, 0:WP], sg[0:96, 0:WP])
                nc.vector.tensor_mul(ot[0:40, WP : 2 * WP], sg[0:40, WP : 2 * WP],
                                     sg[0:40, WP : 2 * WP])
                nc.sync.dma_start(y[b, 0, 0:96, :], ot[0:96, 0:WP])
                nc.sync.dma_start(y[b, 0, 96:136, :], ot[0:40, WP : 2 * WP])

            # ---- software-pipelined schedule over 2 images x 8 chunks
            for blk in range(3):
                emit_gray(0, blk)
            pooled = {0: None, 1: None}
            pooled[0] = outp.tile([128, 2 * WP], F32, tag="pooled", name="pooled0")
            xys = {}
            slots = [(b, j) for b in range(B_LOC) for j in range(N_WC)]
            for s, (b, j) in enumerate(slots):
                if (b, j) == (0, 5):
                    emit_gray(1, 0)
                if (b, j) == (0, 6):
                    emit_gray(1, 1)
                if (b, j) == (0, 7):
                    emit_gray(1, 2)
                if j == 0 and b == 1:
                    pooled[1] = outp.tile([128, 2 * WP], F32, tag="pooled", name="pooled1")
                xys[(b, j)] = stage_a(b, j, s)
                if s >= 1:
                    pb, pj = slots[s - 1]
                    stage_b(pb, pj, xys.pop((pb, pj)), pooled[pb])
                    if pj == N_WC - 1:
                        finish_image(pb, pooled[pb])
            pb, pj = slots[-1]
            stage_b(pb, pj, xys.pop((pb, pj)), pooled[pb])
            finish_image(pb, pooled[pb])

    split_multi_waits(nc)
    return nc


_NC = None
_CONSTS = None
TRACE = False
LAST_EXEC_NS = None


def kernel(**inputs):
    global _NC, _CONSTS, LAST_EXEC_NS
    left_rgb = np.ascontiguousarray(np.asarray(inputs["left_rgb"], dtype=np.float32))
    assert left_rgb.shape == (B_FULL, C, H, W)
    if _NC is None:
        _NC = build_module()
        _CONSTS = build_constants()
    band_a, band_b, p4, cv = _CONSTS
    in_maps = [
        {
            "x": np.ascontiguousarray(left_rgb[i * B_LOC : (i + 1) * B_LOC]),
            "bA": band_a,
            "bB": band_b,
            "p4": p4,
            "cv": cv,
        }
        for i in range(N_CORES)
    ]
    res = run_bass_kernel_spmd(
        _NC, in_maps, core_ids=list(range(N_CORES)), trace=TRACE
    )
    LAST_EXEC_NS = res.exec_time_ns
    out = np.empty((B_FULL, 1, HP, WP), dtype=np.float32)
    for i in range(N_CORES):
        out[i * B_LOC : (i + 1) * B_LOC] = res.results[i]["y"]
    return out


# revision 18
# speedup vs baseline: 1.2100x; 1.2100x over previous
"""EdgeGuidance Trainium2 kernel (v3).

Pipeline per image [3,544,960] -> [1,136,240]:
  gray = w.RGB  ->  smooth = gauss5x5(reflect)  ->  gx,gy = sobel(zero-pad)
  mag = sqrt(gx^2+gy^2+1e-6)  ->  4x4 avgpool  ->  sigmoid(5(x-0.2))^2

All linear steps fold into two banded-matrix passes on the PE (bf16
operands, fp32 PSUM):  gx = A_x @ gray @ Bx^T,  gy = A_y @ gray @ By^T.
Phase A uses gray as stationary so its output lands transposed ([w,s]);
phase B contracts over w with the B^T band as stationary.

Accuracy: gray is stored CENTERED (gray - c) in bf16, which also keeps
the phase-A output small in bf16. The centering constant re-enters as
rank-1 terms c*(B@1)(A@1)^T; the sobel bands' row-sums vanish in the
interior, so the correction is nonzero only on boundary rows/cols of
gx/gy and is applied with tiny tensor_adds into PSUM.

Input DMA: only full-128-partition 2-level loads (these spray across
all 16 SDMA engines; partial-partition or deeper patterns collapse
onto 2 engines/ring). Rows [0,256) and [236,492) as pair-of-row tiles
[128,1920], rows [416,544) as [128,960].

PSUM: single-bank tile rings (psa x4, psb x3, pooled x1 = 8 banks)
with chunk-staggered issue order so all engines pipeline.

Data parallel over batch: 8 cores x 2 images.
"""

import numpy as np
import ml_dtypes

import concourse.bass as bass
import concourse.tile as tile
from concourse import mybir
from concourse.bass_utils import run_bass_kernel_spmd

F32 = mybir.dt.float32
BF16 = mybir.dt.bfloat16
AF = mybir.ActivationFunctionType
ALU = mybir.AluOpType

B_FULL, C, H, W = 16, 3, 544, 960
N_CORES = 8
B_LOC = B_FULL // N_CORES
HP, WP = H // 4, W // 4  # 136, 240

BLUR_K, SIGMA = 5, 1.5
W_R, W_G, W_B = 0.2989, 0.587, 0.114
CEN = 1.672  # gray centering constant

SB = [(0, 250), (250, 486), (486, 544)]  # s-blocks; widths 250, 236, 58
N_WC = 8
BA_COLS = 2060


def _wj(j):
    return max(0, 120 * j - 3), min(W, 120 * j + 123)


# ---------------------------------------------------------------- numpy bands
def _blur1d():
    x = np.arange(BLUR_K, dtype=np.float64) - (BLUR_K - 1) / 2.0
    g = np.exp(-(x**2) / (2.0 * SIGMA**2))
    return g / g.sum()


def _band_reflect(n, taps):
    r = len(taps) // 2
    m = np.zeros((n, n), dtype=np.float64)
    for s in range(n):
        for d in range(-r, r + 1):
            i = s + d
            if i < 0:
                i = -i
            elif i >= n:
                i = 2 * n - 2 - i
            m[s, i] += taps[d + r]
    return m


def _band_zero(n, taps):
    r = len(taps) // 2
    m = np.zeros((n, n), dtype=np.float64)
    for s in range(n):
        for d in range(-r, r + 1):
            i = s + d
            if 0 <= i < n:
                m[s, i] += taps[d + r]
    return m


def _bands():
    g1 = _blur1d()
    vb = _band_reflect(H, g1)
    hb = _band_reflect(W, g1)
    ax = _band_zero(H, [1.0, 2.0, 1.0]) @ vb * W_R
    ay = _band_zero(H, [-1.0, 0.0, 1.0]) @ vb * W_R
    bx = _band_zero(W, [-1.0, 0.0, 1.0]) @ hb
    by = _band_zero(W, [1.0, 2.0, 1.0]) @ hb
    return ax, ay, bx, by


def build_constants():
    ax, ay, bx, by = _bands()

    band_a = np.zeros((128, BA_COLS), dtype=np.float64)
    # sb0: P0 rows 0..255 in pairs (q = row parity)
    s0, s1 = SB[0]
    w = s1 - s0
    for q in range(2):
        for p in range(128):
            r = 2 * p + q
            band_a[p, 500 * q : 500 * q + w] = ax[s0:s1, r]
            band_a[p, 500 * q + w : 500 * q + 2 * w] = ay[s0:s1, r]
    # sb1: P1 rows 236..491 in pairs
    s0, s1 = SB[1]
    w = s1 - s0
    for q in range(2):
        for p in range(128):
            r = 236 + 2 * p + q
            band_a[p, 1000 + 472 * q : 1000 + 472 * q + w] = ax[s0:s1, r]
            band_a[p, 1000 + 472 * q + w : 1000 + 472 * q + 2 * w] = ay[s0:s1, r]
    # sb2: P2 rows 416..543 (singles)
    s0, s1 = SB[2]
    w = s1 - s0
    for p in range(128):
        r = 416 + p
        band_a[p, 1944 : 1944 + w] = ax[s0:s1, r]
        band_a[p, 1944 + w : 1944 + 2 * w] = ay[s0:s1, r]

    band_b = np.zeros((128, 2 * N_WC * 120), dtype=np.float64)
    for t, m in enumerate((bx, by)):
        for j in range(N_WC):
            w0, w1 = _wj(j)
            blk = m[120 * j : 120 * j + 120, w0:w1].T
            band_b[0 : w1 - w0, (t * N_WC + j) * 120 : (t * N_WC + j + 1) * 120] = blk

    p4 = np.zeros((128, 30), dtype=np.float64)
    for wp in range(120):
        p4[wp, wp // 4] = 1.0 / 16.0

    # rank-1 centering corrections folded into the phase-B contraction:
    # band_b row 127 carries c*u (row-sums of B bands); xy row 127 carries
    # the v row-sums (of A bands), so the matmul adds c*u[w']*v[s].
    ux, vx = bx.sum(axis=1), ax.sum(axis=1)  # ux: +1 at w'=0, -1 at w'=959
    uy, vy = by.sum(axis=1), ay.sum(axis=1)  # vy: +W_R at s=0, -W_R at s=543
    for t, u in enumerate((ux, uy)):
        for j in range(N_WC):
            band_b[127, (t * N_WC + j) * 120 : (t * N_WC + j + 1) * 120] = (
                CEN * u[120 * j : 120 * j + 120]
            )
    cv = np.zeros((128, 1088), dtype=np.float32)
    for i, (s0, s1) in enumerate(SB):
        o = (0, 500, 972)[i]
        w = s1 - s0
        cv[127, o : o + w] = vx[s0:s1]
        cv[127, o + w : o + 2 * w] = vy[s0:s1]

    bf = ml_dtypes.bfloat16
    return band_a.astype(bf), band_b.astype(bf), p4.astype(bf), cv


# ------------------------------------------------------------------ bass build
def split_multi_waits(nc):
    """walrus in this container only accepts 1 sync-wait per instruction;
    hoist extra waits onto preceding same-engine NoOps."""
    for fn in nc.m.functions:
        for bb in fn.blocks:
            new_list, changed = [], False
            for ins in bb.instructions:
                si = ins.sync_info
                waits = list(si.on_wait) if si is not None else []
                if len(waits) > 1:
                    changed = True
                    for i, wt in enumerate(waits[:-1]):
                        new_list.append(
                            mybir.InstNoOp(
                                name=f"{ins.name}_ws{i}",
                                engine=ins.engine,
                                bass_nofuse=True,
                                sync_info=mybir.SyncInfo(on_wait=[wt], on_update=[]),
                            )
                        )
                    si.on_wait = [waits[-1]]
                    ins.sync_info = si
                new_list.append(ins)
            if changed:
                bb.instructions = new_list


def build_module():
    nc = bass.Bass("TRN2", target_bir_lowering=False, debug=False)
    x = nc.dram_tensor("x", [B_LOC, C, H, W], F32, kind="ExternalInput").ap()
    ba = nc.dram_tensor("bA", [128, BA_COLS], BF16, kind="ExternalInput").ap()
    bb_ = nc.dram_tensor("bB", [128, 2 * N_WC * 120], BF16, kind="ExternalInput").ap()
    p4 = nc.dram_tensor("p4", [128, 30], BF16, kind="ExternalInput").ap()
    cv = nc.dram_tensor("cv", [128, 1088], F32, kind="ExternalInput").ap()
    y = nc.dram_tensor("y", [B_LOC, 1, HP, WP], F32, kind="ExternalOutput").ap()
    xf = x.rearrange("b c h w -> (b c) (h w)")

    dmae = [nc.sync, nc.scalar]

    with tile.TileContext(nc) as tc:
        with (
            tc.tile_pool(name="const", bufs=1) as cpool,
            tc.tile_pool(name="rgb", bufs=1) as rgbp,
            tc.tile_pool(name="t1", bufs=2) as t1p,
            tc.tile_pool(name="gray", bufs=1) as grayp,
            tc.tile_pool(name="xy", bufs=3) as xyp,
            tc.tile_pool(name="sq", bufs=2) as sqp,
            tc.tile_pool(name="outp", bufs=2) as outp,
            tc.tile_pool(name="psa", bufs=4, space="PSUM") as psa,
            tc.tile_pool(name="psb", bufs=4, space="PSUM") as psb,
        ):
            # ---- constants
            ba_t = cpool.tile([128, BA_COLS], BF16, tag="ba")
            nc.scalar.dma_start(ba_t[:], ba[:])
            bb_t = cpool.tile([128, 2 * N_WC * 120], BF16, tag="bb")
            nc.scalar.dma_start(bb_t[:], bb_[:])
            p4_t = cpool.tile([128, 30], BF16, tag="p4")
            nc.scalar.dma_start(p4_t[:], p4[:])
            cv_t = cpool.tile([128, 1088], F32, tag="cv")
            nc.scalar.dma_start(cv_t[:], cv[:])
            bias_eps = cpool.tile([128, 1], F32, tag="beps")
            nc.gpsimd.memset(bias_eps[:], 1e-6)
            bias_m1 = cpool.tile([128, 1], F32, tag="bm1")
            nc.gpsimd.memset(bias_m1[:], -1.0)

            # ---- input DMAs: SWDGE (gpsimd) f32->bf16 cast loads, all
            # full-128-partition 2-level patterns (they spray across engines)
            rgb = {}
            for b in range(B_LOC):
                for blk, (r0, r1, fw) in enumerate(
                    [(0, 256, 1920), (236, 492, 1920), (416, 544, 960)]
                ):
                    for c in range(C):
                        t = rgbp.tile([128, fw], BF16, tag=f"rgb{b}{blk}{c}")
                        nc.gpsimd.dma_start(
                            t[:],
                            xf[3 * b + c, r0 * W : r1 * W].rearrange(
                                "(p f) -> p f", f=fw
                            ),
                        )
                        rgb[(b, blk, c)] = t

            gray = {}

            def emit_gray(b, blk):
                # centered gray: g = (R - c) + (wG/wR) G + (wB/wR) B, all bf16
                # via 4x tensor_scalar and 2x tensor_tensor ops (no stt: it
                # has no bf16 2x uop)
                tr, tg, tb = (rgb[(b, blk, c)] for c in range(C))
                fw = 960 if blk == 2 else 1920
                with nc.allow_low_precision(reason="bf16 centered gray"):
                    nc.vector.tensor_scalar_sub(tr[:], tr[:], CEN)
                    t1 = t1p.tile([128, 1920], BF16, tag="t1")
                    nc.vector.tensor_scalar_mul(t1[0:128, 0:fw], tg[:], W_G / W_R)
                    nc.vector.tensor_scalar_mul(tb[:], tb[:], W_B / W_R)
                    nc.vector.tensor_add(t1[0:128, 0:fw], t1[0:128, 0:fw], tr[:])
                    g = grayp.tile([128, fw], BF16, tag=f"g{b}{blk}")
                    nc.vector.tensor_add(g[:], t1[0:128, 0:fw], tb[:])
                gray[(b, blk)] = g

            def stage_a(b, j, slot):
                g0, g1, g2 = gray[(b, 0)], gray[(b, 1)], gray[(b, 2)]
                w0, w1 = _wj(j)
                mj = w1 - w0
                a0 = psa.tile([128, 512], F32, tag="psa")
                nc.tensor.matmul(a0[0:mj, 0:500], g0[0:128, w0:w1],
                                 ba_t[0:128, 0:500], start=True, stop=False)
                nc.tensor.matmul(a0[0:mj, 0:500], g0[0:128, 960 + w0 : 960 + w1],
                                 ba_t[0:128, 500:1000], start=False, stop=True)
                a1 = psa.tile([128, 512], F32, tag="psa")
                nc.tensor.matmul(a1[0:mj, 0:472], g1[0:128, w0:w1],
                                 ba_t[0:128, 1000:1472], start=True, stop=False)
                nc.tensor.matmul(a1[0:mj, 0:472], g1[0:128, 960 + w0 : 960 + w1],
                                 ba_t[0:128, 1472:1944], start=False, stop=True)
                a2 = psa.tile([128, 512], F32, tag="psa")
                nc.tensor.matmul(a2[0:mj, 0:116], g2[0:128, w0:w1],
                                 ba_t[0:128, 1944:2060], start=True, stop=True)
                xy = xyp.tile([128, 1088], BF16, tag="xy")
                if slot < 3:
                    # rows 96:126 zeros + correction row 127; later ring reuses
                    # keep row 127 (drains only touch rows [0:126))
                    nc.vector.tensor_copy(xy[96:128, :], cv_t[96:128, :])
                nc.vector.tensor_copy(xy[0:mj, 0:500], a0[0:mj, 0:500])
                nc.scalar.copy(xy[0:mj, 500:972], a1[0:mj, 0:472])
                nc.vector.tensor_copy(xy[0:mj, 972:1088], a2[0:mj, 0:116])
                return xy

            def stage_b(b, j, xy, pooled):
                bTx = bb_t[0:128, (0 * N_WC + j) * 120 : (0 * N_WC + j + 1) * 120]
                bTy = bb_t[0:128, (1 * N_WC + j) * 120 : (1 * N_WC + j + 1) * 120]
                # b0: gx s[0:486]; b1: gy s[0:486]
                # b2: gx s[486:544] | gy s[486:544] | pool1 | pool2
                b0 = psb.tile([128, 512], F32, tag="psb")
                nc.tensor.matmul(b0[0:120, 0:250], bTx, xy[0:128, 0:250],
                                 start=True, stop=True)
                nc.tensor.matmul(b0[0:120, 250:486], bTx, xy[0:128, 500:736],
                                 start=True, stop=True)
                b1 = psb.tile([128, 512], F32, tag="psb")
                nc.tensor.matmul(b1[0:120, 0:250], bTy, xy[0:128, 250:500],
                                 start=True, stop=True)
                nc.tensor.matmul(b1[0:120, 250:486], bTy, xy[0:128, 736:972],
                                 start=True, stop=True)
                b2 = psb.tile([128, 512], F32, tag="psb")
                nc.tensor.matmul(b2[0:120, 0:58], bTx, xy[0:128, 972:1030],
                                 start=True, stop=True)
                nc.tensor.matmul(b2[0:120, 64:122], bTy, xy[0:128, 1030:1088],
                                 start=True, stop=True)

                # squares (f32), packed to s-order [120, 544]
                sqx = sqp.tile([128, 544], F32, tag="sqx")
                nc.scalar.activation(sqx[0:120, 0:486], b0[0:120, 0:486], AF.Square)
                nc.scalar.activation(sqx[0:120, 486:544], b2[0:120, 0:58], AF.Square)
                sqy = sqp.tile([128, 544], F32, tag="sqy")
                nc.scalar.activation(sqy[0:120, 0:486], b1[0:120, 0:486], AF.Square)
                nc.scalar.activation(sqy[0:120, 486:544], b2[0:120, 64:122], AF.Square)

                m2 = sqp.tile([128, 544], F32, tag="m2")
                nc.gpsimd.tensor_add(m2[0:120, :], sqx[0:120, :], sqy[0:120, :])
                mag = sqp.tile([128, 544], BF16, tag="mag")
                with nc.allow_low_precision(reason="bf16 magnitude"):
                    nc.scalar.activation(
                        mag[0:120, :], m2[0:120, :], AF.Sqrt, bias=bias_eps[0:120, :]
                    )
                    sp = sqp.tile([128, 136], BF16, tag="sp")
                    nc.vector.tensor_reduce(
                        sp[0:120, :],
                        mag[0:120, :].rearrange("p (g f) -> p g f", f=4),
                        axis=mybir.AxisListType.X,
                        op=ALU.add,
                    )
                nc.tensor.matmul(b2[0:96, 128:158], sp[0:120, 0:96], p4_t[0:120, :],
                                 start=True, stop=True)
                nc.tensor.matmul(b2[0:40, 192:222], sp[0:120, 96:136], p4_t[0:120, :],
                                 start=True, stop=True)
                nc.vector.tensor_copy(pooled[0:96, 30 * j : 30 * (j + 1)],
                                      b2[0:96, 128:158])
                nc.scalar.copy(pooled[0:40, WP + 30 * j : WP + 30 * (j + 1)],
                               b2[0:40, 192:222])

            def finish_image(b, pooled):
                sg = outp.tile([128, 2 * WP], F32, tag="sg")
                nc.scalar.activation(sg[0:96, 0:WP], pooled[0:96, 0:WP], AF.Sigmoid,
                                     bias=bias_m1[0:96, :], scale=5.0)
                nc.scalar.activation(sg[0:40, WP : 2 * WP],
                                     pooled[0:40, WP : 2 * WP], AF.Sigmoid,
                                     bias=bias_m1[0:40, :], scale=5.0)
                ot = outp.tile([128, 2 * WP], F32, tag="ot")
                nc.vector.tensor_mul(ot[0:96, 0:WP], sg[0:96, 0:WP], sg[0:96, 0:WP])
                nc.vector.tensor_mul(ot[0:40, WP : 2 * WP], sg[0:40, WP : 2 * WP],
                                     sg[0:40, WP : 2 * WP])
                nc.sync.dma_start(y[b, 0, 0:96, :], ot[0:96, 0:WP])
                nc.sync.dma_start(y[b, 0, 96:136, :], ot[0:40, WP : 2 * WP])

            # ---- software-pipelined schedule over 2 images x 8 chunks
            for blk in range(3):
                emit_gray(0, blk)
            pooled = {0: None, 1: None}
            pooled[0] = outp.tile([128, 2 * WP], F32, tag="pooled", name="pooled0")
            xys = {}
            slots = [(b, j) for b in range(B_LOC) for j in range(N_WC)]
            for s, (b, j) in enumerate(slots):
                if (b, j) == (0, 5):
                    emit_gray(1, 0)
                if (b, j) == (0, 6):
                    emit_gray(1, 1)
                if (b, j) == (0, 7):
                    emit_gray(1, 2)
                if j == 0 and b == 1:
                    pooled[1] = outp.tile([128, 2 * WP], F32, tag="pooled", name="pooled1")
                xys[(b, j)] = stage_a(b, j, s)
                if s >= 1:
                    pb, pj = slots[s - 1]
                    stage_b(pb, pj, xys.pop((pb, pj)), pooled[pb])
                    if pj == N_WC - 1:
                        finish_image(pb, pooled[pb])
            pb, pj = slots[-1]
            stage_b(pb, pj, xys.pop((pb, pj)), pooled[pb])
            finish_image(pb, pooled[pb])

    split_multi_waits(nc)
    return nc


_NC = None
_CONSTS = None
TRACE = False
LAST_EXEC_NS = None


def kernel(**inputs):
    global _NC, _CONSTS, LAST_EXEC_NS
    left_rgb = np.ascontiguousarray(np.asarray(inputs["left_rgb"], dtype=np.float32))
    assert left_rgb.shape == (B_FULL, C, H, W)
    if _NC is None:
        _NC = build_module()
        _CONSTS = build_constants()
    band_a, band_b, p4, cv = _CONSTS
    in_maps = [
        {
            "x": np.ascontiguousarray(left_rgb[i * B_LOC : (i + 1) * B_LOC]),
            "bA": band_a,
            "bB": band_b,
            "p4": p4,
            "cv": cv,
        }
        for i in range(N_CORES)
    ]
    res = run_bass_kernel_spmd(
        _NC, in_maps, core_ids=list(range(N_CORES)), trace=TRACE
    )
    LAST_EXEC_NS = res.exec_time_ns
    out = np.empty((B_FULL, 1, HP, WP), dtype=np.float32)
    for i in range(N_CORES):
        out[i * B_LOC : (i + 1) * B_LOC] = res.results[i]["y"]
    return out
